# revision 6
# baseline (speedup 1.0000x reference)
"""LocalRNN (sliding-window GRU) Trainium2 Bass kernel.

Problem: x:[8,2048,768] f32, GRU weights w_ih/w_hh:[768,2304], biases:[2304].
For every position t, run a ksize=8-step GRU over the window x[t-7..t]
(zero left-pad) and emit the final hidden state -> [8,2048,768].

Strategy
--------
Batch (8) is sharded 1:1 across the 8 NeuronCores; weights replicated.
Everything on-chip lives TRANSPOSED ([feature, time]) so the per-step
recurrent matmul  gh.T = w_hh.T @ h.T  needs no transposes in the loop:

  phase 1:  X.T via PE transposes; GX = w_ih.T @ X.T + (b_ih [+ b_hh])
            streamed to DRAM scratch ([3D, T+7]; r/z gates in bf16 with
            b_hh pre-folded, n gate in fp32r).
  step 0:   h1 = (1-z0)*n0 from GX only (h0 = 0: no matmul).
  steps 1-7: per (chunk of 512 t, d-tile of 128 feats):
            PSUM[r] = sum_k w_hh_r[k].T @ H[k] + I@GX_r   (fp32r MMs,
            PSUM[z] likewise, PSUM[n] without the GX identity-MM)
            r  = sigmoid(PSUM_r)            (ACT, bias folded via GX)
            zb = sigmoid(-PSUM_z)  (= 1-z)  (ACT, scale=-1)
            g1 = (PSUM_n + b_hh_n) * r      (DVE scalar_tensor_tensor)
            g1 += GX_n ; n = tanh(g1)       (DVE + ACT)
            e  = zb * (n - H)               (DVE x2)
            -- after all 6 d-tiles of the chunk (their MMs read full H):
            H += e                          (DVE, writes fp32r)
  phase 3:  PE-transpose H back to [t, d] rows, DMA out.

All matmuls run as fp32r (TF32-like, 1 cyc/row, ~1.5e-4 rel err measured
on HW) - 4x faster than fp32.
"""

import sys
import time

import numpy as np

sys.path.insert(0, "/opt/trn_rl_repo")

import concourse.bass as bass  # noqa: E402
import concourse.tile as tile  # noqa: E402
from concourse import bacc, mybir  # noqa: E402
from concourse.masks import make_identity  # noqa: E402

F32 = mybir.dt.float32
F32R = mybir.dt.float32r
BF16 = mybir.dt.bfloat16
FP16 = mybir.dt.float16
RZ_DT = FP16  # storage dtype for r/z GX (fp16: 10 mantissa bits)
AF = mybir.ActivationFunctionType
OP = mybir.AluOpType

D = 768
G3 = 3 * D
KD = D // 128          # 6 k-tiles
M = G3 // 128          # 18 m-tiles (0-5 r, 6-11 z, 12-17 n)


def build(T=2048, KSIZE=8, CHUNK=512, repeat=1):
    NCH = T // CHUNK
    TP = T + KSIZE - 1
    SUB = CHUNK // 128       # 128-row subtiles per chunk

    nc = bacc.Bacc("TRN2", target_bir_lowering=False, debug=False)
    x = nc.dram_tensor("x", [T, D], F32, kind="ExternalInput").ap()
    w_ih = nc.dram_tensor("w_ih", [D, G3], F32R, kind="ExternalInput").ap()
    w_hh = nc.dram_tensor("w_hh", [D, G3], F32R, kind="ExternalInput").ap()
    b_ih = nc.dram_tensor("b_ih", [G3], F32, kind="ExternalInput").ap()
    b_hh = nc.dram_tensor("b_hh", [G3], F32, kind="ExternalInput").ap()
    out = nc.dram_tensor("out", [T, D], F32, kind="ExternalOutput").ap()
    gxrz_d = nc.dram_tensor("gxrz", [2 * D, TP], RZ_DT, kind="Internal").ap()
    gxn_d = nc.dram_tensor("gxn", [D, TP], F32R, kind="Internal").ap()

    with tile.TileContext(nc) as tc:
        with tc.tile_pool(name="perm", bufs=1) as perm:
            ident_f = perm.tile([128, 128], F32, name="ident_f")
            make_identity(nc, ident_f[:])
            ident_r = perm.tile([128, 128], F32R, name="ident_r")
            nc.vector.tensor_copy(ident_r[:], ident_f[:])
            ident_b = perm.tile([128, 128], RZ_DT, name="ident_b")
            nc.vector.tensor_copy(ident_b[:], ident_f[:])

            bih_sb = perm.tile([128, M], F32, name="bih")
            nc.sync.dma_start(bih_sb[:], b_ih.rearrange("(m p) -> p m", p=128))
            bhh_sb = perm.tile([128, M], F32, name="bhh")
            nc.sync.dma_start(bhh_sb[:], b_hh.rearrange("(m p) -> p m", p=128))
            bsum = perm.tile([128, M], F32, name="bsum")
            nc.vector.tensor_tensor(bsum[:], bih_sb[:], bhh_sb[:], op=OP.add)
            negbhh = perm.tile([128, M], F32, name="negbhh")
            nc.vector.tensor_scalar_mul(negbhh[:], bhh_sb[:], -1.0)

            whh_r = []
            for k in range(KD):
                w = perm.tile([128, G3], F32R, name=f"whh{k}")
                nc.sync.dma_start(w[:], w_hh[k * 128:(k + 1) * 128, :])
                whh_r.append(w)

            for rep in range(repeat):
                _emit_once(
                    nc, tc, rep, T, KSIZE, CHUNK, NCH, TP, SUB,
                    x, w_ih, out, gxrz_d, gxn_d,
                    ident_f, ident_r, ident_b, bih_sb, bhh_sb, bsum, negbhh,
                    whh_r,
                )

    nc.compile()
    return nc


def _emit_once(nc, tc, rep, T, KSIZE, CHUNK, NCH, TP, SUB,
               x, w_ih, out, gxrz_d, gxn_d,
               ident_f, ident_r, ident_b, bih_sb, bhh_sb, bsum, negbhh,
               whh_r):
    PAD = KSIZE - 1

    # ---------------- phase 1: GX = w_ih.T @ X.T + biases -> DRAM ----------
    with (
        tc.tile_pool(name=f"wih{rep}", bufs=1) as wihp,
        tc.tile_pool(name=f"xload{rep}", bufs=2) as xp,
        tc.tile_pool(name=f"xt{rep}", bufs=2) as xtp,
        tc.tile_pool(name=f"gxstage{rep}", bufs=3) as stp,
        tc.tile_pool(name=f"pst{rep}", bufs=2, space="PSUM") as ps_t,
        tc.tile_pool(name=f"psg{rep}", bufs=2, space="PSUM") as ps_g,
    ):
        wih_r = []
        for k in range(KD):
            w = wihp.tile([128, G3], F32R, name=f"wih{k}")
            nc.sync.dma_start(w[:], w_ih[k * 128:(k + 1) * 128, :])
            wih_r.append(w)

        # left-pad region: gx = bias only (zero input)
        zt = stp.tile([128, PAD], F32, name="padzero")
        nc.vector.memset(zt[:], 0.0)
        for m in range(M):
            if m < 12:
                pad = stp.tile([128, PAD], RZ_DT, name="padrz")
                nc.vector.tensor_scalar(pad[:], zt[:], bsum[:, m:m + 1], None, op0=OP.add)
                nc.sync.dma_start(gxrz_d[m * 128:(m + 1) * 128, 0:PAD], pad[:])
            else:
                pad = stp.tile([128, PAD], F32R, name="padn")
                nc.vector.tensor_scalar(pad[:], zt[:], bih_sb[:, m:m + 1], None, op0=OP.add)
                nc.sync.dma_start(gxn_d[(m - 12) * 128:(m - 11) * 128, 0:PAD], pad[:])

        for c in range(NCH):
            xts = [xtp.tile([128, CHUNK], F32R, name=f"xt{k}") for k in range(KD)]
            for i in range(SUB):
                xn = xp.tile([128, D], F32, name="xn")
                t0 = c * CHUNK + i * 128
                nc.sync.dma_start(xn[:], x[t0:t0 + 128, :])
                for k in range(KD):
                    pt = ps_t.tile([128, 128], F32, name="pt")
                    nc.tensor.transpose(pt[:], xn[:, k * 128:(k + 1) * 128], ident_f[:])
                    nc.scalar.activation(
                        xts[k][:, i * 128:(i + 1) * 128], pt[:], AF.Copy
                    )
            for m in range(M):
                pg = ps_g.tile([128, CHUNK], F32, name="pg")
                for k in range(KD):
                    nc.tensor.matmul(
                        pg[:], wih_r[k][:, m * 128:(m + 1) * 128], xts[k][:],
                        start=(k == 0), stop=(k == KD - 1),
                    )
                lo = PAD + c * CHUNK
                if m < 12:
                    st = stp.tile([128, CHUNK], RZ_DT, name="strz")
                    nc.vector.tensor_scalar(st[:], pg[:], bsum[:, m:m + 1], None, op0=OP.add)
                    nc.sync.dma_start(gxrz_d[m * 128:(m + 1) * 128, lo:lo + CHUNK], st[:])
                else:
                    st = stp.tile([128, CHUNK], F32R, name="stn")
                    nc.vector.tensor_scalar(st[:], pg[:], bih_sb[:, m:m + 1], None, op0=OP.add)
                    nc.sync.dma_start(gxn_d[(m - 12) * 128:(m - 11) * 128, lo:lo + CHUNK], st[:])

    # ---------------- phase 2: the 8 GRU steps -----------------------------
    gxrz_v = gxrz_d.rearrange("(m p) t -> p m t", p=128)   # [128, 12, TP]
    gxn_v = gxn_d.rearrange("(m p) t -> p m t", p=128)     # [128, 6, TP]

    with (
        tc.tile_pool(name=f"H{rep}", bufs=1) as hp,
        tc.tile_pool(name=f"grz{rep}", bufs=2) as grzp,
        tc.tile_pool(name=f"gn{rep}", bufs=2) as gnp,
        tc.tile_pool(name=f"tmp{rep}", bufs=2) as tp2,
        tc.tile_pool(name=f"ee{rep}", bufs=2 * KD) as eep,
        tc.tile_pool(name=f"ost{rep}", bufs=2) as ostp,
        tc.tile_pool(name=f"ps2{rep}", bufs=2, space="PSUM") as ps2,
        tc.tile_pool(name=f"pso{rep}", bufs=2, space="PSUM") as ps_o,
    ):
        H = [hp.tile([128, T], F32R, name=f"H{k}") for k in range(KD)]

        for j in range(KSIZE):
            for c in range(NCH):
                cs = slice(c * CHUNK, (c + 1) * CHUNK)
                # stream this step's GX window into SBUF
                grz = grzp.tile([128, 12 * CHUNK], RZ_DT, name="grz")
                nc.sync.dma_start(
                    grz[:].rearrange("p (m t) -> p m t", t=CHUNK),
                    gxrz_v[:, :, j + c * CHUNK: j + (c + 1) * CHUNK],
                )
                gn = gnp.tile([128, 6 * CHUNK], F32R, name="gn")
                nc.sync.dma_start(
                    gn[:].rearrange("p (m t) -> p m t", t=CHUNK),
                    gxn_v[:, :, j + c * CHUNK: j + (c + 1) * CHUNK],
                )

                def grz_m(m):
                    return grz[:, m * CHUNK:(m + 1) * CHUNK]

                def gn_f32(dd):
                    return gn[:, dd * CHUNK:(dd + 1) * CHUNK].bitcast(F32)

                es = []
                for d in range(KD):
                    if j == 0:
                        # h0 = 0: r/z pre-activations are GX alone (b_ih+b_hh
                        # were already folded into the rz GX in phase 1)
                        r = tp2.tile([128, CHUNK], F32, name="r")
                        nc.scalar.activation(r[:], grz_m(d), AF.Sigmoid)
                        zb = tp2.tile([128, CHUNK], F32, name="zb")
                        nc.scalar.activation(
                            zb[:], grz_m(d + 6), AF.Sigmoid, scale=-1.0
                        )
                        g1 = tp2.tile([128, CHUNK], F32, name="g1")
                        nc.vector.scalar_tensor_tensor(
                            g1[:], r[:], bhh_sb[:, d + 12:d + 13], gn_f32(d),
                            op0=OP.mult, op1=OP.add,
                        )
                        n = tp2.tile([128, CHUNK], F32, name="n")
                        nc.scalar.activation(n[:], g1[:], AF.Tanh)
                        # h1 = (1-z)*n, written as fp32r
                        nc.vector.tensor_tensor(H[d][:, cs], zb[:], n[:], op=OP.mult)
                        continue

                    pr = ps2.tile([128, CHUNK], F32, name="pr")
                    pz = ps2.tile([128, CHUNK], F32, name="pz")
                    pn = ps2.tile([128, CHUNK], F32, name="pn")
                    for ps, m in ((pr, d), (pz, d + 6), (pn, d + 12)):
                        for k in range(KD):
                            nc.tensor.matmul(
                                ps[:], whh_r[k][:, m * 128:(m + 1) * 128],
                                H[k][:, cs],
                                start=(k == 0),
                                stop=(m >= 12 and k == KD - 1),
                            )
                    nc.tensor.matmul(pr[:], ident_b[:], grz_m(d), start=False, stop=True)
                    nc.tensor.matmul(pz[:], ident_b[:], grz_m(d + 6), start=False, stop=True)

                    r = tp2.tile([128, CHUNK], F32, name="r")
                    nc.scalar.activation(r[:], pr[:], AF.Sigmoid)
                    zb = tp2.tile([128, CHUNK], F32, name="zb")
                    nc.scalar.activation(zb[:], pz[:], AF.Sigmoid, scale=-1.0)
                    g1 = tp2.tile([128, CHUNK], F32, name="g1")
                    nc.vector.scalar_tensor_tensor(
                        g1[:], pn[:], bhh_sb[:, d + 12:d + 13], r[:],
                        op0=OP.add, op1=OP.mult,
                    )
                    nc.vector.tensor_tensor(g1[:], g1[:], gn_f32(d), op=OP.add)
                    n = tp2.tile([128, CHUNK], F32, name="n")
                    nc.scalar.activation(n[:], g1[:], AF.Tanh)
                    e = eep.tile([128, CHUNK], F32, name="e")
                    nc.vector.tensor_tensor(e[:], n[:], H[d][:, cs].bitcast(F32), op=OP.subtract)
                    nc.vector.tensor_tensor(e[:], zb[:], e[:], op=OP.mult)
                    es.append((d, e))

                # deferred H update: all of this chunk's matmuls read old H
                for d, e in es:
                    nc.vector.tensor_tensor(
                        H[d][:, cs], H[d][:, cs].bitcast(F32), e[:], op=OP.add
                    )

                # ---------------- phase 3: transpose H chunk -> out --------
                if j == KSIZE - 1:
                    for i in range(SUB):
                        t0 = c * CHUNK + i * 128
                        og = ostp.tile([128, D], F32, name="og")
                        for dd in range(KD):
                            po = ps_o.tile([128, 128], F32, name="po")
                            nc.tensor.transpose(
                                po[:], H[dd][:, t0:t0 + 128].bitcast(F32), ident_f[:]
                            )
                            nc.scalar.activation(
                                og[:, dd * 128:(dd + 1) * 128], po[:], AF.Copy
                            )
                        nc.sync.dma_start(out[t0:t0 + 128, :], og[:])


# --------------------------------------------------------------------------
# PJRT runner (resident buffers, jit built once)
# --------------------------------------------------------------------------
class BassRunner:
    def __init__(self, nc, n_cores: int):
        import jax
        from jax.sharding import Mesh, PartitionSpec
        from jax.experimental.shard_map import shard_map
        from concourse.bass2jax import (
            _bass_exec_p, install_neuronx_cc_hook, partition_id_tensor,
        )

        install_neuronx_cc_hook()
        self.jax = jax
        self.nc = nc
        self.n_cores = n_cores

        partition_name = (
            nc.partition_id_tensor.name if nc.partition_id_tensor else None
        )
        in_names, out_names, out_avals, zero_outs = [], [], [], []
        for alloc in nc.m.functions[0].allocations:
            if not isinstance(alloc, mybir.MemoryLocationSet):
                continue
            name = alloc.memorylocations[0].name
            if alloc.kind == "ExternalInput":
                if name != partition_name:
                    in_names.append(name)
            elif alloc.kind == "ExternalOutput":
                shape = tuple(alloc.tensor_shape)
                dtype = mybir.dt.np(alloc.dtype)
                out_names.append(name)
                out_avals.append(jax.core.ShapedArray(shape, dtype))
                zero_outs.append(np.zeros(shape, dtype))
        self.in_names = in_names
        self.out_names = out_names
        self.zero_outs = zero_outs
        n_params = len(in_names)
        all_in_names = list(in_names) + list(out_names)
        if partition_name is not None:
            all_in_names.append(partition_name)

        def _body(*args):
            operands = list(args)
            if partition_name is not None:
                operands.append(partition_id_tensor())
            outs = _bass_exec_p.bind(
                *operands,
                out_avals=tuple(out_avals),
                in_names=tuple(all_in_names),
                out_names=tuple(out_names),
                lowering_input_output_aliases=(),
                sim_require_finite=True,
                sim_require_nnan=True,
                nc=nc,
            )
            return tuple(outs)

        devices = jax.devices()[:n_cores]
        assert len(devices) == n_cores, (
            f"need {n_cores} neuron devices, have {len(jax.devices())}"
        )
        if n_cores == 1:
            self.fn = jax.jit(_body, keep_unused=True)
        else:
            mesh = Mesh(np.asarray(devices), ("core",))
            in_specs = (PartitionSpec("core"),) * (n_params + len(out_names))
            out_specs = (PartitionSpec("core"),) * len(out_names)
            self.fn = jax.jit(
                shard_map(_body, mesh=mesh, in_specs=in_specs,
                          out_specs=out_specs, check_rep=False),
                keep_unused=True,
            )
        self._dev_args = None

    def stage(self, in_maps):
        assert len(in_maps) == self.n_cores
        if self.n_cores == 1:
            concat = [np.asarray(in_maps[0][n]) for n in self.in_names]
            concat += list(self.zero_outs)
        else:
            concat = [
                np.concatenate([np.asarray(m[n]) for m in in_maps], axis=0)
                for n in self.in_names
            ]
            concat += [
                np.concatenate([z] * self.n_cores, axis=0) for z in self.zero_outs
            ]
        self._dev_args = self.jax.device_put(concat)
        self.jax.block_until_ready(self._dev_args)

    def run(self):
        outs = self.fn(*self._dev_args)
        self.jax.block_until_ready(outs)
        return outs

    def run_results(self):
        outs = self.run()
        per_core = [{} for _ in range(self.n_cores)]
        for name, arr in zip(self.out_names, outs):
            arr = np.asarray(arr)
            if self.n_cores == 1:
                per_core[0][name] = arr
            else:
                for c, s in enumerate(np.split(arr, self.n_cores, axis=0)):
                    per_core[c][name] = s
        return per_core

    def time_runs(self, iters=10, warmup=2):
        for _ in range(warmup):
            self.run()
        ts = []
        for _ in range(iters):
            t0 = time.perf_counter()
            self.run()
            ts.append(time.perf_counter() - t0)
        return ts


# --------------------------------------------------------------------------
# public entry point
# --------------------------------------------------------------------------
_CACHE = {}


def _get_runner(T, KSIZE, n_cores, repeat=1):
    key = (T, KSIZE, n_cores, repeat)
    if key not in _CACHE:
        nc = build(T=T, KSIZE=KSIZE, repeat=repeat)
        _CACHE[key] = BassRunner(nc, n_cores)
    return _CACHE[key]


def kernel(x, w_ih, w_hh, b_ih, b_hh, ksize):
    x = np.ascontiguousarray(np.asarray(x, dtype=np.float32))
    B, T, _D = x.shape
    ksize = int(ksize)
    runner = _get_runner(T, ksize, B)
    w_ih = np.ascontiguousarray(np.asarray(w_ih, dtype=np.float32))
    w_hh = np.ascontiguousarray(np.asarray(w_hh, dtype=np.float32))
    b_ih = np.ascontiguousarray(np.asarray(b_ih, dtype=np.float32))
    b_hh = np.ascontiguousarray(np.asarray(b_hh, dtype=np.float32))
    in_maps = [
        {"x": x[b], "w_ih": w_ih, "w_hh": w_hh, "b_ih": b_ih, "b_hh": b_hh}
        for b in range(B)
    ]
    runner.stage(in_maps)
    res = runner.run_results()
    return np.stack([res[b]["out"] for b in range(B)], axis=0)
